# revision 6
# baseline (speedup 1.0000x reference)
"""GCN layer (gather -> x@W -> normalized scatter-add -> bias -> PReLU) on 8 trn2 cores.

Strategy (node sharding):
  - 100000 nodes padded to 102400 = 8 * 12800; core c owns nodes [c*12800, (c+1)*12800).
  - Phase 1: each core computes hs = dinv * (x_own @ W) for its nodes (fp32 on PE, x tiles
    transposed via PE), written as bf16 in 4 quarter tensors (3200 rows each).
  - Phase 2: 4 AllGathers (one per quarter) -> 4 shared tables [25600, 128] bf16; each
    pipelines behind the quarter's phase-1 writes and ahead of phase-3 consumers.
  - Phase 3: edges sorted by destination; destinations processed in 128-node windows
    (PSUM [128 dst, 128 feat], 8 windows in flight); per 128-edge block a one-hot
    S [edge, dst] is built on DVE (iota == reldst) and PE accumulates psum += S^T @ G,
    where G = dma_gather'ed hs rows (int16 idx into the 25600-row quarter table).
    Self-loops are one identity matmul per window on contiguous hs rows.
    Epilogue: out = prelu(dinv_dst * psum + b) with per-feature alpha.
"""
import sys
sys.path.insert(0, '/opt/trn_rl_repo')

import numpy as np
import ml_dtypes

N = 100000
NCORES = 8
SH = 12800                 # nodes per core
NP = NCORES * SH           # 102400 padded nodes
H = 128                    # output features
KIN = 256                  # input features
WIN = 128                  # dst window size
NW = SH // WIN             # 100 windows per core
WG = 8                     # windows per PSUM group
NG = (NW + WG - 1) // WG   # 13 groups (last has 4)
NQ = 4                     # source quarters
QSH = SH // NQ             # 3200 rows of own shard per quarter
QT = QSH // WIN            # 25 tiles per quarter
TAB = NCORES * QSH         # 25600 rows per gather table (int16-safe)
XB = 5                     # phase-1 tiles per DMA batch (25 tiles/quarter = 5 batches)

bf16 = ml_dtypes.bfloat16


def _preprocess(edge_index):
    src = np.asarray(edge_index[0]).astype(np.int64)
    dst = np.asarray(edge_index[1]).astype(np.int64)
    E = src.shape[0]

    deg = (np.bincount(dst, minlength=N) + 1).astype(np.float32)
    dinv = (1.0 / np.sqrt(deg)).astype(np.float32)
    dinv_np = np.ones(NP, np.float32)
    dinv_np[:N] = dinv

    core = dst // SH
    w_in_core = (dst % SH) // WIN            # 0..NW-1
    g = w_in_core // WG
    wi = w_in_core % WG
    # source quarter + row in its gather table
    s_core = src // SH
    s_li = src % SH
    q = s_li // QSH
    tab_row = s_core * QSH + (s_li % QSH)    # < TAB

    key = ((core * NG + g) * NQ + q) * WG + wi
    nbins_pc = NG * NQ * WG
    order = np.argsort(key, kind='stable')
    o_tab = tab_row[order]
    o_dst = dst[order]
    o_key = key[order]

    cnt_all = np.bincount(key, minlength=NCORES * nbins_pc)
    bin_start = np.concatenate([[0], np.cumsum(cnt_all)])[:-1]
    rank = np.arange(E, dtype=np.int64) - bin_start[o_key]

    cnt = cnt_all.reshape(NCORES, NG, NQ, WG)
    nblk = np.ceil(cnt.max(axis=0) / WIN).astype(np.int64)   # [NG, NQ, WG] common
    pad_sizes = (nblk * WIN).reshape(-1)
    offs = np.concatenate([[0], np.cumsum(pad_sizes)])
    TOT = int(offs[-1])
    NBLK = TOT // WIN

    bin_in_core = o_key % nbins_pc
    pos = offs[bin_in_core] + rank
    win_base = core * SH + w_in_core * WIN
    o_win_base = win_base[order]
    o_core = core[order]

    per_core = []
    for c in range(NCORES):
        m = o_core == c
        idxq = np.zeros(TOT, np.int16)
        rels = np.full(TOT, -1.0, np.float32)
        p_c = pos[m]
        idxq[p_c] = o_tab[m].astype(np.int16)
        rels[p_c] = (o_dst[m] - o_win_base[m]).astype(np.float32)
        idx16 = np.tile(np.ascontiguousarray(idxq.reshape(TOT // 16, 16).T), (8, 1))
        relm = np.ascontiguousarray(rels.reshape(NBLK, WIN).T)   # [128, NBLK]
        dinv_own = np.ascontiguousarray(
            dinv_np[c * SH:(c + 1) * SH].reshape(NW, WIN).T)     # [128, NW]
        per_core.append(dict(idx16=idx16, reldst=relm, dinv=dinv_own))

    calls = []          # (g, q, off_idx, nidx, [(Bcol, w), ...])
    Bcol = 0
    last_block_of_win = {}
    for gg in range(NG):
        for qq in range(NQ):
            blocks = []
            off_idx = None
            for wii in range(WG):
                nb = int(nblk[gg, qq, wii])
                if nb == 0:
                    continue
                w = gg * WG + wii
                if w >= NW:
                    continue
                bin_i = (gg * NQ + qq) * WG + wii
                if off_idx is None:
                    off_idx = int(offs[bin_i])
                for k in range(nb):
                    blocks.append((Bcol, w))
                    last_block_of_win[w] = Bcol
                    Bcol += 1
            if blocks:
                calls.append((gg, qq, off_idx, len(blocks) * WIN, blocks))
    sched = dict(calls=calls, last_block=last_block_of_win, NBLK=NBLK, TOT=TOT)
    return sched, per_core, dinv_np


def _build(sched):
    import os
    from concourse import bass, bacc, tile, mybir
    from concourse.masks import make_identity

    nc = bacc.Bacc("TRN2", target_bir_lowering=False, debug=False,
                   enable_asserts=True, num_devices=NCORES)

    x_d = nc.dram_tensor("x_own", [SH, KIN], mybir.dt.float32, kind="ExternalInput")
    w_d = nc.dram_tensor("w_mat", [KIN, H], mybir.dt.float32, kind="ExternalInput")
    b_d = nc.dram_tensor("b_vec", [H], mybir.dt.float32, kind="ExternalInput")
    a_d = nc.dram_tensor("a_vec", [H], mybir.dt.float32, kind="ExternalInput")
    dinv_d = nc.dram_tensor("dinv_own", [128, NW], mybir.dt.float32, kind="ExternalInput")
    idx_d = nc.dram_tensor("idx16", [128, sched["TOT"] // 16], mybir.dt.int16, kind="ExternalInput")
    rel_d = nc.dram_tensor("reldst", [128, sched["NBLK"]], mybir.dt.float32, kind="ExternalInput")

    out_d = nc.dram_tensor("out_own", [SH, H], mybir.dt.float32, kind="ExternalOutput")

    hs_q = [nc.dram_tensor(f"hs_q{k}", [QSH, H], mybir.dt.bfloat16) for k in range(NQ)]
    hs_tab = [nc.dram_tensor(f"hs_tab{k}", [TAB, H], mybir.dt.bfloat16, addr_space="Shared")
              for k in range(NQ)]

    calls = sched["calls"]
    last_block = sched["last_block"]
    max_call_blk = max(len(cb[4]) for cb in calls)
    nblk_of_group = [sum(len(cb[4]) for cb in calls if cb[0] == gg) for gg in range(NG)]
    first_col_of_group = [min([cb[4][0][0] for cb in calls if cb[0] == gg] or [0])
                          for gg in range(NG)]

    with tile.TileContext(nc) as tc:
        with tc.tile_pool(name="consts", bufs=1) as cp, tc.tile_pool(name="sb", bufs=3) as sb:
            # ---------------- constants ----------------
            iota_i = cp.tile([128, 128], mybir.dt.int32)
            nc.gpsimd.iota(iota_i[:], pattern=[[1, 128]], base=0, channel_multiplier=0)
            iota_f = cp.tile([128, 128], mybir.dt.float32)
            nc.vector.tensor_copy(iota_f[:], iota_i[:])

            ident_f = cp.tile([128, 128], mybir.dt.float32)
            make_identity(nc, ident_f[:])
            ident_b = cp.tile([128, 128], mybir.dt.bfloat16)
            nc.vector.tensor_copy(ident_b[:], ident_f[:])

            w0 = cp.tile([128, H], mybir.dt.float32)
            w1 = cp.tile([128, H], mybir.dt.float32)
            nc.sync.dma_start(w0[:], w_d[0:128, :])
            nc.sync.dma_start(w1[:], w_d[128:256, :])

            dinv_sb = cp.tile([128, NW], mybir.dt.float32)
            nc.sync.dma_start(dinv_sb[:], dinv_d[:, :])

            ones1 = cp.tile([1, H], mybir.dt.float32)
            nc.vector.memset(ones1[:], 1.0)
            bvec = cp.tile([1, H], mybir.dt.float32)
            nc.sync.dma_start(bvec[:], b_d[None, :])
            avec = cp.tile([1, H], mybir.dt.float32)
            nc.sync.dma_start(avec[:], a_d[None, :])

            b128 = cp.tile([128, H], mybir.dt.float32)
            a128 = cp.tile([128, H], mybir.dt.float32)

            with tc.tile_pool(name="psum1", bufs=1, space="PSUM") as pp1:
                bc_ps = pp1.tile([128, H], mybir.dt.float32, space="PSUM", tag="bc", bufs=1)
                nc.tensor.matmul(out=bc_ps[:], lhsT=ones1[:], rhs=bvec[:], start=True, stop=True)
                nc.vector.tensor_copy(b128[:], bc_ps[:])
                ac_ps = pp1.tile([128, H], mybir.dt.float32, space="PSUM", tag="bc", bufs=1)
                nc.tensor.matmul(out=ac_ps[:], lhsT=ones1[:], rhs=avec[:], start=True, stop=True)
                nc.vector.tensor_copy(a128[:], ac_ps[:])

                # ---------------- phase 1 (+ per-quarter AllGather) ----------------
                for bb in range(NW // XB):          # batches of XB tiles
                    qk = bb // (QT // XB)           # quarter of this batch
                    t0 = bb * XB
                    x_t = sb.tile([128, XB * KIN], mybir.dt.float32, tag="x_t", bufs=2)
                    nc.sync.dma_start(
                        x_t[:],
                        x_d[t0 * 128:(t0 + XB) * 128, :].rearrange(
                            "(t p) k -> p t k", p=128))
                    hs_b = sb.tile([128, XB * H], mybir.dt.bfloat16, tag="hs_b", bufs=2)
                    for tt in range(XB):
                        i = t0 + tt
                        h_ps = pp1.tile([128, H], mybir.dt.float32, space="PSUM",
                                        tag="h_ps", bufs=3)
                        for kk in range(2):
                            xt_ps = pp1.tile([128, 128], mybir.dt.float32, space="PSUM",
                                             tag="xt_ps", bufs=4)
                            nc.tensor.transpose(
                                xt_ps[:], x_t[:, (tt * 2 + kk) * 128:(tt * 2 + kk + 1) * 128],
                                ident_f[:])
                            xt_sb = sb.tile([128, 128], mybir.dt.float32, tag="xt_sb", bufs=4)
                            nc.scalar.activation(xt_sb[:], xt_ps[:],
                                                 mybir.ActivationFunctionType.Copy)
                            nc.tensor.matmul(out=h_ps[:], lhsT=xt_sb[:],
                                             rhs=(w0 if kk == 0 else w1)[:],
                                             start=(kk == 0), stop=(kk == 1))
                        nc.vector.tensor_scalar(out=hs_b[:, tt * H:(tt + 1) * H], in0=h_ps[:],
                                                scalar1=dinv_sb[:, i:i + 1], scalar2=None,
                                                op0=mybir.AluOpType.mult)
                    r0 = t0 * 128 - qk * QSH
                    nc.sync.dma_start(
                        hs_q[qk][r0:r0 + XB * 128, :].rearrange("(t p) k -> p t k", p=128),
                        hs_b[:])
                    if (bb + 1) % (QT // XB) == 0:
                        nc.gpsimd.collective_compute(
                            "AllGather", mybir.AluOpType.bypass,
                            replica_groups=[list(range(NCORES))],
                            ins=[hs_q[qk].ap().opt()],
                            outs=[hs_tab[qk].ap().opt()],
                        )

            # ---------------- phase 3 ----------------
            with tc.tile_pool(name="psum3", bufs=WG, space="PSUM") as pp3:
                for gg in range(NG):
                    wlo = gg * WG
                    whi = min(wlo + WG, NW)
                    nwin = whi - wlo
                    # batched self-loop loads, split at quarter boundaries
                    self_g = sb.tile([128, WG * H], mybir.dt.bfloat16, tag="self_g", bufs=2)
                    wseg = wlo
                    while wseg < whi:
                        k = wseg // QT
                        wend = min(whi, (k + 1) * QT)
                        r0 = (wseg - k * QT) * 128
                        nc.sync.dma_start(
                            self_g[:, (wseg - wlo) * H:(wend - wlo) * H],
                            hs_q[k][r0:r0 + (wend - wseg) * 128, :].rearrange(
                                "(t p) k -> p t k", p=128))
                        wseg = wend

                    pw = {}
                    for w in range(wlo, whi):
                        pw[w] = pp3.tile([128, H], mybir.dt.float32, space="PSUM",
                                         tag="pw", name=f"pw{w}")
                        nc.tensor.matmul(out=pw[w][:], lhsT=ident_b[:],
                                         rhs=self_g[:, (w - wlo) * H:(w - wlo + 1) * H],
                                         start=True, stop=(w not in last_block))

                    if nblk_of_group[gg]:
                        rd_sb = sb.tile([128, max(nblk_of_group)], mybir.dt.float32,
                                        tag="rd", bufs=2)
                        c0 = first_col_of_group[gg]
                        nc.sync.dma_start(rd_sb[:, 0:nblk_of_group[gg]],
                                          rel_d[:, c0:c0 + nblk_of_group[gg]])

                    for (g_c, qq, off_idx, nidx, blocks) in calls:
                        if g_c != gg:
                            continue
                        idx_sb = sb.tile([128, max_call_blk * 8], mybir.dt.int16,
                                         tag="idx", bufs=3)
                        nc.sync.dma_start(idx_sb[:, 0:nidx // 16],
                                          idx_d[:, off_idx // 16: (off_idx + nidx) // 16])
                        g_t = sb.tile([128, max_call_blk, H], mybir.dt.bfloat16,
                                      tag="g_t", bufs=3)
                        nc.gpsimd.dma_gather(
                            g_t[:, 0:nidx // 128, :], hs_tab[qq][:, :],
                            idx_sb[:, 0:nidx // 16], nidx, nidx, H,
                            single_packet=False)
                        for (bcol, w) in blocks:
                            s_t = sb.tile([128, 128], mybir.dt.bfloat16, tag="s_t", bufs=6)
                            lc = bcol - first_col_of_group[gg]
                            nc.vector.tensor_scalar(
                                out=s_t[:], in0=iota_f[:],
                                scalar1=rd_sb[:, lc:lc + 1], scalar2=None,
                                op0=mybir.AluOpType.is_equal)
                            slot = (bcol - blocks[0][0])
                            nc.tensor.matmul(out=pw[w][:], lhsT=s_t[:], rhs=g_t[:, slot, :],
                                             start=False, stop=(last_block.get(w) == bcol))

                    # epilogue, batched output DMA per group
                    o_g = sb.tile([128, WG * H], mybir.dt.float32, tag="o_g", bufs=2)
                    for w in range(wlo, whi):
                        u = sb.tile([128, H], mybir.dt.float32, tag="u", bufs=3)
                        nc.scalar.activation(u[:], pw[w][:], mybir.ActivationFunctionType.Copy,
                                             scale=dinv_sb[:, w:w + 1])
                        u2 = sb.tile([128, H], mybir.dt.float32, tag="u2", bufs=3)
                        nc.vector.tensor_tensor(out=u2[:], in0=u[:], in1=b128[:],
                                                op=mybir.AluOpType.add)
                        r2 = sb.tile([128, H], mybir.dt.float32, tag="r2", bufs=3)
                        nc.scalar.activation(r2[:], u2[:], mybir.ActivationFunctionType.Relu,
                                             scale=-1.0)
                        m = sb.tile([128, H], mybir.dt.float32, tag="m", bufs=3)
                        nc.gpsimd.tensor_tensor(out=m[:], in0=r2[:], in1=a128[:],
                                                op=mybir.AluOpType.mult)
                        r1 = sb.tile([128, H], mybir.dt.float32, tag="r1", bufs=3)
                        nc.scalar.activation(r1[:], u2[:], mybir.ActivationFunctionType.Relu)
                        nc.vector.tensor_tensor(out=o_g[:, (w - wlo) * H:(w - wlo + 1) * H],
                                                in0=r1[:], in1=m[:],
                                                op=mybir.AluOpType.subtract)
                    nc.sync.dma_start(
                        out_d[wlo * 128:whi * 128, :].rearrange("(t p) k -> p t k", p=128),
                        o_g[:, 0:nwin * H])

    nc.compile()
    return nc


def kernel(x, edge_index, W, b, alpha):
    from concourse.bass_utils import run_bass_kernel_spmd

    x = np.asarray(x, dtype=np.float32)
    W = np.asarray(W, dtype=np.float32)
    b = np.asarray(b, dtype=np.float32)
    alpha = np.asarray(alpha, dtype=np.float32)

    sched, per_core, dinv_np = _preprocess(edge_index)
    nc = _build(sched)

    x_pad = np.zeros((NP, KIN), np.float32)
    x_pad[:N] = x

    in_maps = []
    for c in range(NCORES):
        in_maps.append({
            "x_own": np.ascontiguousarray(x_pad[c * SH:(c + 1) * SH]),
            "w_mat": W, "b_vec": b, "a_vec": alpha,
            "dinv_own": per_core[c]["dinv"],
            "idx16": per_core[c]["idx16"],
            "reldst": per_core[c]["reldst"],
        })

    res = run_bass_kernel_spmd(nc, in_maps, core_ids=list(range(NCORES)))
    out = np.concatenate([res.results[c]["out_own"] for c in range(NCORES)], axis=0)
    return np.ascontiguousarray(out[:N])
